# revision 11
# baseline (speedup 1.0000x reference)
"""Trainium2 Bass kernel for single-head cross-modal attention.

Problem: B=8, S=2048, D=1024 (fp32 inputs)
    q = image_emb @ Wq.T + bq
    k = text_emb  @ Wk.T + bk
    v = text_emb  @ Wv.T + bv
    out = softmax(q @ k.T / sqrt(D)) @ v

Sharding: data-parallel over batch — core b handles batch element b.

Per-core dataflow (all matmuls bf16 with fp32 PSUM accumulation):
  - X^T tiles produced by DMA-transpose (bf16, zero PE cost).
  - Projections computed directly in the layouts the attention matmuls
    need: QT/KT in [e, s] layout, V in natural [s, e] layout.
  - scores^T [k_part, q_free] = KT_tile.T @ QT, so exp(scores)^T is
    directly the stationary operand of the P@V matmul: the 2048x2048
    probability matrix is never transposed on chip.
  - softmax without max-subtraction (scores ~ N(0,1), |s| <= ~6: exp is
    safe in fp32).  Row sums come from an extra ones-column appended to
    V (softmax denominators emerge as one extra N=1 matmul column that
    shares the stationary operand with the P@V matmuls).
  - final normalize fused: out = (att_psum * recip) + bv_bcast in one
    DVE op per 512-wide chunk.
"""

import sys
import os

for _p in ("/opt/trn_rl_repo", "/root/.axon_site/_ro/trn_rl_repo"):
    if os.path.isdir(_p) and _p not in sys.path:
        sys.path.insert(0, _p)

import numpy as np
import ml_dtypes

import concourse.bass as bass
import concourse.mybir as mybir
import concourse.tile as tile
from concourse import bacc
from concourse.bass_utils import run_bass_kernel_spmd

BF16 = mybir.dt.bfloat16
F32 = mybir.dt.float32
AF = mybir.ActivationFunctionType
ALU = mybir.AluOpType

B, S, D = 8, 2048, 1024
P = 128
ND = D // P          # 8  d/e tiles
NS = S // P          # 16 s tiles
QC = 512             # q chunk width (matmul free dim / PSUM bank)
NQC = S // QC        # 4
EC = 512             # e chunk width for V / output
SCALE = 1.0 / float(np.sqrt(D))

_CACHE = {}


def _build_nc():
    nc = bacc.Bacc("TRN2", target_bir_lowering=False, debug=False, num_devices=8)

    xi_d = nc.dram_tensor("xiT", [D, S], BF16, kind="ExternalInput").ap()
    xt_d = nc.dram_tensor("xtT", [D, S], BF16, kind="ExternalInput").ap()
    wqt_d = nc.dram_tensor("wqt", [D, D], BF16, kind="ExternalInput").ap()  # Wq.T
    wkt_d = nc.dram_tensor("wkt", [D, D], BF16, kind="ExternalInput").ap()
    wvt_d = nc.dram_tensor("wvt", [D, D], BF16, kind="ExternalInput").ap()
    bq_d = nc.dram_tensor("bq", [D], F32, kind="ExternalInput").ap()
    bk_d = nc.dram_tensor("bk", [D], F32, kind="ExternalInput").ap()
    bv_d = nc.dram_tensor("bv", [D], F32, kind="ExternalInput").ap()
    out_d = nc.dram_tensor("out", [S, D], F32, kind="ExternalOutput").ap()

    with tile.TileContext(nc) as tc:
        _emit(nc, tc, xi_d, xt_d, wqt_d, wkt_d, wvt_d, bq_d, bk_d, bv_d, out_d)
    nc.compile()
    return nc


def _emit(nc, tc, xi_d, xt_d, wqt_d, wkt_d, wvt_d, bq_d, bk_d, bv_d, out_d):
    NH = QC // P  # 4 q_tiles per chunk
    with (
        tc.tile_pool(name="const", bufs=1) as pc,
        tc.tile_pool(name="qkv", bufs=1) as pqkv,
    ):
        # persistent activations
        qt = pqkv.tile([P, ND, S], BF16, name="qt", tag="qt")    # QT[e,q]
        kt = pqkv.tile([P, ND, S], BF16, name="kt", tag="kt")    # KT[e,k]
        v = pqkv.tile([P, NS, D], BF16, name="v", tag="v")       # V[s,e]

        # constants
        bias_q = pc.tile([P, ND], F32, name="bias_q", tag="bias_q")
        bias_k = pc.tile([P, ND], F32, name="bias_k", tag="bias_k")
        ones_row = pc.tile([1, P], F32, name="ones_row", tag="ones_row")
        bv_row = pc.tile([1, D], F32, name="bv_row", tag="bv_row")
        bv_bcast = pc.tile([P, D], F32, name="bv_bcast", tag="bv_bcast")
        ones_col = pc.tile([P, 1], BF16, name="ones_col", tag="ones_col")

        with (
            tc.tile_pool(name="w", bufs=1) as pw,
            tc.tile_pool(name="xs", bufs=3) as pxs,
            tc.tile_pool(name="psP", bufs=4, space="PSUM") as psP,
        ):
            wk_sb = pw.tile([P, ND, D], BF16, name="wk_sb", tag="wk_sb")
            wv_sb = pw.tile([P, ND, D], BF16, name="wv_sb", tag="wv_sb")
            wq_sb = pw.tile([P, ND, D], BF16, name="wq_sb", tag="wq_sb")
            nc.gpsimd.dma_start(bias_k[:], bk_d.rearrange("(t p) -> p t", p=P))
            nc.gpsimd.dma_start(bias_q[:], bq_d.rearrange("(t p) -> p t", p=P))
            nc.gpsimd.dma_start(bv_row[:], bv_d[None, :])
            for d in range(ND):
                nc.gpsimd.dma_start(wk_sb[:, d, :], wkt_d[d * P:(d + 1) * P, :])
            for d in range(ND):
                nc.gpsimd.dma_start(wv_sb[:, d, :], wvt_d[d * P:(d + 1) * P, :])
            for d in range(ND):
                nc.gpsimd.dma_start(wq_sb[:, d, :], wqt_d[d * P:(d + 1) * P, :])
            nc.vector.memset(ones_row[:], 1.0)
            nc.vector.memset(ones_col[:], 1.0)

            # --- KT[e_t, kc] = sum_d Wk^T[d, e_t].T @ XtT[d, kc]  (+ bk) ---
            for qc in range(NQC):
                xc = pxs.tile([P, ND, QC], BF16, name="xc", tag="xs")
                for d in range(ND):
                    eng = nc.sync if d % 2 == 0 else nc.scalar
                    eng.dma_start(
                        xc[:, d, :],
                        xt_d[d * P:(d + 1) * P, qc * QC:(qc + 1) * QC])
                for et in range(ND):
                    ps = psP.tile([P, QC], F32, name="ps", tag="ps")
                    for d in range(ND):
                        nc.tensor.matmul(
                            ps[:], wk_sb[:, d, et * P:(et + 1) * P], xc[:, d, :],
                            start=(d == 0), stop=(d == ND - 1))
                    nc.vector.tensor_scalar_add(
                        kt[:, et, qc * QC:(qc + 1) * QC], ps[:],
                        bias_k[:, et:et + 1])

            # --- V[s_t, e] = sum_d XtT[d, s_t].T @ Wv^T[d, e] ---
            for vc in range(NQC):
                xc = pxs.tile([P, ND, QC], BF16, name="xc", tag="xs")
                for d in range(ND):
                    eng = nc.sync if d % 2 == 0 else nc.scalar
                    eng.dma_start(
                        xc[:, d, :],
                        xt_d[d * P:(d + 1) * P, vc * QC:(vc + 1) * QC])
                for si in range(NH):
                    st = vc * NH + si
                    ps0 = psP.tile([P, EC], F32, name="ps0", tag="ps")
                    ps1 = psP.tile([P, EC], F32, name="ps1", tag="ps")
                    for d in range(ND):
                        lhs = xc[:, d, si * P:(si + 1) * P]
                        nc.tensor.matmul(ps0[:], lhs, wv_sb[:, d, 0:EC],
                                         start=(d == 0), stop=(d == ND - 1))
                        nc.tensor.matmul(ps1[:], lhs, wv_sb[:, d, EC:D],
                                         start=(d == 0), stop=(d == ND - 1))
                    nc.vector.tensor_copy(v[:, st, 0:EC], ps0[:])
                    nc.vector.tensor_copy(v[:, st, EC:D], ps1[:])

            # bv broadcast (independent; fills scheduling gaps)
            for c in range(2):
                pb = psP.tile([P, EC], F32, name="pb", tag="ps")
                nc.tensor.matmul(
                    pb[:], ones_row[:], bv_row[:, c * EC:(c + 1) * EC],
                    start=True, stop=True)
                nc.vector.tensor_copy(bv_bcast[:, c * EC:(c + 1) * EC], pb[:])

            # --- QT[e_t, qc] = sum_d Wq^T[d, e_t].T @ XiT[d, qc]  (+ bq) ---
            for qc in range(NQC):
                xc = pxs.tile([P, ND, QC], BF16, name="xc", tag="xs")
                for d in range(ND):
                    eng = nc.sync if d % 2 == 0 else nc.scalar
                    eng.dma_start(
                        xc[:, d, :],
                        xi_d[d * P:(d + 1) * P, qc * QC:(qc + 1) * QC])
                for et in range(ND):
                    ps = psP.tile([P, QC], F32, name="ps", tag="ps")
                    for d in range(ND):
                        nc.tensor.matmul(
                            ps[:], wq_sb[:, d, et * P:(et + 1) * P], xc[:, d, :],
                            start=(d == 0), stop=(d == ND - 1))
                    nc.vector.tensor_scalar_add(
                        qt[:, et, qc * QC:(qc + 1) * QC], ps[:],
                        bias_q[:, et:et + 1])

        # --- attention ---
        with (
            tc.tile_pool(name="et", bufs=2) as pet,
            tc.tile_pool(name="outp", bufs=3) as pout,
            tc.tile_pool(name="stat", bufs=4) as pstat,
            tc.tile_pool(name="psST", bufs=2, space="PSUM") as psST,
            tc.tile_pool(name="psAV", bufs=3, space="PSUM") as psAV,
            tc.tile_pool(name="psRS", bufs=2, space="PSUM") as psRS,
        ):
            for qc in range(NQC):
                # scores^T for this q chunk: ET[kk, q] = exp(scale*KT.T@QT)
                et_t = pet.tile([P, NS, QC], BF16, name="et_t", tag="et")
                for kk in range(NS):
                    st_ps = psST.tile([P, QC], F32, name="st_ps", tag="st")
                    for e in range(ND):
                        nc.tensor.matmul(
                            st_ps[:],
                            kt[:, e, kk * P:(kk + 1) * P],
                            qt[:, e, qc * QC:(qc + 1) * QC],
                            start=(e == 0), stop=(e == ND - 1))
                    nc.scalar.activation(et_t[:, kk, :], st_ps[:], AF.Exp,
                                         scale=SCALE)

                # attended[q_t, :] = (ET.T @ V) * recip + bv
                for qs in range(NH):
                    a0 = psAV.tile([P, EC], F32, name="a0", tag="av")
                    a1 = psAV.tile([P, EC], F32, name="a1", tag="av")
                    rs = psRS.tile([P, 1], F32, name="rs", tag="rs")
                    for kk in range(NS):
                        lhs = et_t[:, kk, qs * P:(qs + 1) * P]
                        nc.tensor.matmul(a0[:], lhs, v[:, kk, 0:EC],
                                         start=(kk == 0), stop=(kk == NS - 1))
                        nc.tensor.matmul(a1[:], lhs, v[:, kk, EC:D],
                                         start=(kk == 0), stop=(kk == NS - 1))
                        nc.tensor.matmul(rs[:], lhs, ones_col[:],
                                         start=(kk == 0), stop=(kk == NS - 1))
                    recip = pstat.tile([P, 1], F32, name="recip", tag="recip")
                    nc.vector.reciprocal(recip[:], rs[:])
                    ob = pout.tile([P, D], F32, name="ob", tag="ob")
                    nc.vector.scalar_tensor_tensor(
                        ob[:, 0:EC], a0[:], recip[:], bv_bcast[:, 0:EC],
                        op0=ALU.mult, op1=ALU.add)
                    nc.vector.scalar_tensor_tensor(
                        ob[:, EC:D], a1[:], recip[:], bv_bcast[:, EC:D],
                        op0=ALU.mult, op1=ALU.add)
                    q_tile = qc * NH + qs
                    nc.sync.dma_start(
                        out_d[q_tile * P:(q_tile + 1) * P, :], ob[:])


def get_nc():
    if "nc" not in _CACHE:
        _CACHE["nc"] = _build_nc()
    return _CACHE["nc"]


def _prep_inputs(image_emb, text_emb, Wq, bq, Wk, bk, Wv, bv):
    bf = ml_dtypes.bfloat16
    xi = np.asarray(image_emb).astype(bf)   # [B, S, D]
    xt = np.asarray(text_emb).astype(bf)
    xiT = np.ascontiguousarray(xi.transpose(0, 2, 1))  # [B, D, S]
    xtT = np.ascontiguousarray(xt.transpose(0, 2, 1))
    wqt = np.ascontiguousarray(np.asarray(Wq).T).astype(bf)
    wkt = np.ascontiguousarray(np.asarray(Wk).T).astype(bf)
    wvt = np.ascontiguousarray(np.asarray(Wv).T).astype(bf)
    bq = np.asarray(bq, dtype=np.float32)
    bk = np.asarray(bk, dtype=np.float32)
    bv = np.asarray(bv, dtype=np.float32)
    in_maps = []
    for b in range(B):
        in_maps.append({
            "xiT": xiT[b], "xtT": xtT[b],
            "wqt": wqt, "wkt": wkt, "wvt": wvt,
            "bq": bq, "bk": bk, "bv": bv,
        })
    return in_maps


def run(image_emb, text_emb, Wq, bq, Wk, bk, Wv, bv, trace=False, **spmd_kwargs):
    nc = get_nc()
    in_maps = _prep_inputs(image_emb, text_emb, Wq, bq, Wk, bk, Wv, bv)
    res = run_bass_kernel_spmd(nc, in_maps, list(range(B)), trace=trace,
                               **spmd_kwargs)
    out = np.stack([res.results[b]["out"] for b in range(B)], axis=0)
    return out, res


def kernel(image_emb, text_emb, edge_index=None, Wq=None, bq=None, Wk=None,
           bk=None, Wv=None, bv=None, **_unused):
    out, _ = run(image_emb, text_emb, Wq, bq, Wk, bk, Wv, bv, trace=False)
    return out


# revision 12
# speedup vs baseline: 1.0117x; 1.0117x over previous
"""Trainium2 Bass kernel for single-head cross-modal attention.

Problem: B=8, S=2048, D=1024 (fp32 inputs)
    q = image_emb @ Wq.T + bq
    k = text_emb  @ Wk.T + bk
    v = text_emb  @ Wv.T + bv
    out = softmax(q @ k.T / sqrt(D)) @ v

Sharding: data-parallel over batch — core b handles batch element b.

Per-core dataflow (all matmuls bf16 with fp32 PSUM accumulation):
  - X^T tiles produced by DMA-transpose (bf16, zero PE cost).
  - Projections computed directly in the layouts the attention matmuls
    need: QT/KT in [e, s] layout, V in natural [s, e] layout.
  - scores^T [k_part, q_free] = KT_tile.T @ QT, so exp(scores)^T is
    directly the stationary operand of the P@V matmul: the 2048x2048
    probability matrix is never transposed on chip.
  - softmax without max-subtraction (scores ~ N(0,1), |s| <= ~6: exp is
    safe in fp32).  Row sums come from an extra ones-column appended to
    V (softmax denominators emerge as one extra N=1 matmul column that
    shares the stationary operand with the P@V matmuls).
  - final normalize fused: out = (att_psum * recip) + bv_bcast in one
    DVE op per 512-wide chunk.
"""

import sys
import os

for _p in ("/opt/trn_rl_repo", "/root/.axon_site/_ro/trn_rl_repo"):
    if os.path.isdir(_p) and _p not in sys.path:
        sys.path.insert(0, _p)

import numpy as np
import ml_dtypes

import concourse.bass as bass
import concourse.mybir as mybir
import concourse.tile as tile
from concourse import bacc
from concourse.bass_utils import run_bass_kernel_spmd

BF16 = mybir.dt.bfloat16
F32 = mybir.dt.float32
AF = mybir.ActivationFunctionType
ALU = mybir.AluOpType

B, S, D = 8, 2048, 1024
P = 128
ND = D // P          # 8  d/e tiles
NS = S // P          # 16 s tiles
QC = 512             # q chunk width (matmul free dim / PSUM bank)
NQC = S // QC        # 4
EC = 512             # e chunk width for V / output
SCALE = 1.0 / float(np.sqrt(D))

_CACHE = {}


def _build_nc():
    nc = bacc.Bacc("TRN2", target_bir_lowering=False, debug=False, num_devices=8)

    xi_d = nc.dram_tensor("xiT", [D, S], BF16, kind="ExternalInput").ap()
    xt_d = nc.dram_tensor("xtT", [D, S], BF16, kind="ExternalInput").ap()
    wqt_d = nc.dram_tensor("wqt", [D, D], BF16, kind="ExternalInput").ap()  # Wq.T
    wkt_d = nc.dram_tensor("wkt", [D, D], BF16, kind="ExternalInput").ap()
    wvt_d = nc.dram_tensor("wvt", [D, D], BF16, kind="ExternalInput").ap()
    bq_d = nc.dram_tensor("bq", [P, ND], F32, kind="ExternalInput").ap()
    bk_d = nc.dram_tensor("bk", [P, ND], F32, kind="ExternalInput").ap()
    bv_d = nc.dram_tensor("bv", [D], F32, kind="ExternalInput").ap()
    out_d = nc.dram_tensor("out", [S, D], F32, kind="ExternalOutput").ap()

    with tile.TileContext(nc) as tc:
        _emit(nc, tc, xi_d, xt_d, wqt_d, wkt_d, wvt_d, bq_d, bk_d, bv_d, out_d)
    nc.compile()
    return nc


def _emit(nc, tc, xi_d, xt_d, wqt_d, wkt_d, wvt_d, bq_d, bk_d, bv_d, out_d):
    NH = QC // P  # 4 q_tiles per chunk
    with (
        tc.tile_pool(name="const", bufs=1) as pc,
        tc.tile_pool(name="qkv", bufs=1) as pqkv,
    ):
        # persistent activations
        qt = pqkv.tile([P, ND, S], BF16, name="qt", tag="qt")    # QT[e,q]
        kt = pqkv.tile([P, ND, S], BF16, name="kt", tag="kt")    # KT[e,k]
        v = pqkv.tile([P, NS, D], BF16, name="v", tag="v")       # V[s,e]

        # constants
        bias_q = pc.tile([P, ND], F32, name="bias_q", tag="bias_q")
        bias_k = pc.tile([P, ND], F32, name="bias_k", tag="bias_k")
        ones_row = pc.tile([1, P], F32, name="ones_row", tag="ones_row")
        bv_row = pc.tile([1, D], F32, name="bv_row", tag="bv_row")
        bv_bcast = pc.tile([P, D], F32, name="bv_bcast", tag="bv_bcast")
        ones_col = pc.tile([P, 1], BF16, name="ones_col", tag="ones_col")

        with (
            tc.tile_pool(name="w", bufs=1) as pw,
            tc.tile_pool(name="xs", bufs=3) as pxs,
            tc.tile_pool(name="psP", bufs=4, space="PSUM") as psP,
        ):
            wk_sb = pw.tile([P, ND, D], BF16, name="wk_sb", tag="wk_sb")
            wv_sb = pw.tile([P, ND, D], BF16, name="wv_sb", tag="wv_sb")
            wq_sb = pw.tile([P, ND, D], BF16, name="wq_sb", tag="wq_sb")
            for d in range(ND):
                nc.gpsimd.dma_start(wk_sb[:, d, :], wkt_d[d * P:(d + 1) * P, :])
            nc.sync.dma_start(bias_k[:], bk_d[:])
            nc.sync.dma_start(bias_q[:], bq_d[:])
            for d in range(ND):
                nc.gpsimd.dma_start(wv_sb[:, d, :], wvt_d[d * P:(d + 1) * P, :])
            nc.gpsimd.dma_start(bv_row[:], bv_d[None, :])
            for d in range(ND):
                nc.gpsimd.dma_start(wq_sb[:, d, :], wqt_d[d * P:(d + 1) * P, :])
            nc.vector.memset(ones_row[:], 1.0)
            nc.vector.memset(ones_col[:], 1.0)

            # --- KT[e_t, kc] = sum_d Wk^T[d, e_t].T @ XtT[d, kc]  (+ bk) ---
            for qc in range(NQC):
                xc = pxs.tile([P, ND, QC], BF16, name="xc", tag="xs")
                for d in range(ND):
                    eng = nc.sync if d % 2 == 0 else nc.scalar
                    eng.dma_start(
                        xc[:, d, :],
                        xt_d[d * P:(d + 1) * P, qc * QC:(qc + 1) * QC])
                for et in range(ND):
                    ps = psP.tile([P, QC], F32, name="ps", tag="ps")
                    for d in range(ND):
                        nc.tensor.matmul(
                            ps[:], wk_sb[:, d, et * P:(et + 1) * P], xc[:, d, :],
                            start=(d == 0), stop=(d == ND - 1))
                    nc.vector.tensor_scalar_add(
                        kt[:, et, qc * QC:(qc + 1) * QC], ps[:],
                        bias_k[:, et:et + 1])

            # --- V[s_t, e] = sum_d XtT[d, s_t].T @ Wv^T[d, e] ---
            for vc in range(NQC):
                xc = pxs.tile([P, ND, QC], BF16, name="xc", tag="xs")
                for d in range(ND):
                    eng = nc.sync if d % 2 == 0 else nc.scalar
                    eng.dma_start(
                        xc[:, d, :],
                        xt_d[d * P:(d + 1) * P, vc * QC:(vc + 1) * QC])
                for si in range(NH):
                    st = vc * NH + si
                    ps0 = psP.tile([P, EC], F32, name="ps0", tag="ps")
                    ps1 = psP.tile([P, EC], F32, name="ps1", tag="ps")
                    for d in range(ND):
                        lhs = xc[:, d, si * P:(si + 1) * P]
                        nc.tensor.matmul(ps0[:], lhs, wv_sb[:, d, 0:EC],
                                         start=(d == 0), stop=(d == ND - 1))
                        nc.tensor.matmul(ps1[:], lhs, wv_sb[:, d, EC:D],
                                         start=(d == 0), stop=(d == ND - 1))
                    nc.vector.tensor_copy(v[:, st, 0:EC], ps0[:])
                    nc.vector.tensor_copy(v[:, st, EC:D], ps1[:])

            # bv broadcast (independent; fills scheduling gaps)
            for c in range(2):
                pb = psP.tile([P, EC], F32, name="pb", tag="ps")
                nc.tensor.matmul(
                    pb[:], ones_row[:], bv_row[:, c * EC:(c + 1) * EC],
                    start=True, stop=True)
                nc.vector.tensor_copy(bv_bcast[:, c * EC:(c + 1) * EC], pb[:])

            # --- QT[e_t, qc] = sum_d Wq^T[d, e_t].T @ XiT[d, qc]  (+ bq) ---
            for qc in range(NQC):
                xc = pxs.tile([P, ND, QC], BF16, name="xc", tag="xs")
                for d in range(ND):
                    eng = nc.sync if d % 2 == 0 else nc.scalar
                    eng.dma_start(
                        xc[:, d, :],
                        xi_d[d * P:(d + 1) * P, qc * QC:(qc + 1) * QC])
                for et in range(ND):
                    ps = psP.tile([P, QC], F32, name="ps", tag="ps")
                    for d in range(ND):
                        nc.tensor.matmul(
                            ps[:], wq_sb[:, d, et * P:(et + 1) * P], xc[:, d, :],
                            start=(d == 0), stop=(d == ND - 1))
                    nc.vector.tensor_scalar_add(
                        qt[:, et, qc * QC:(qc + 1) * QC], ps[:],
                        bias_q[:, et:et + 1])

        # --- attention ---
        with (
            tc.tile_pool(name="et", bufs=2) as pet,
            tc.tile_pool(name="outp", bufs=3) as pout,
            tc.tile_pool(name="stat", bufs=4) as pstat,
            tc.tile_pool(name="psST", bufs=2, space="PSUM") as psST,
            tc.tile_pool(name="psAV", bufs=3, space="PSUM") as psAV,
            tc.tile_pool(name="psRS", bufs=2, space="PSUM") as psRS,
        ):
            for qc in range(NQC):
                # scores^T for this q chunk: ET[kk, q] = exp(scale*KT.T@QT)
                et_t = pet.tile([P, NS, QC], BF16, name="et_t", tag="et")
                for kk in range(NS):
                    st_ps = psST.tile([P, QC], F32, name="st_ps", tag="st")
                    for e in range(ND):
                        nc.tensor.matmul(
                            st_ps[:],
                            kt[:, e, kk * P:(kk + 1) * P],
                            qt[:, e, qc * QC:(qc + 1) * QC],
                            start=(e == 0), stop=(e == ND - 1))
                    nc.scalar.activation(et_t[:, kk, :], st_ps[:], AF.Exp,
                                         scale=SCALE)

                # attended[q_t, :] = (ET.T @ V) * recip + bv
                for qs in range(NH):
                    a0 = psAV.tile([P, EC], F32, name="a0", tag="av")
                    a1 = psAV.tile([P, EC], F32, name="a1", tag="av")
                    rs = psRS.tile([P, 1], F32, name="rs", tag="rs")
                    for kk in range(NS):
                        lhs = et_t[:, kk, qs * P:(qs + 1) * P]
                        nc.tensor.matmul(a0[:], lhs, v[:, kk, 0:EC],
                                         start=(kk == 0), stop=(kk == NS - 1))
                        nc.tensor.matmul(a1[:], lhs, v[:, kk, EC:D],
                                         start=(kk == 0), stop=(kk == NS - 1))
                        nc.tensor.matmul(rs[:], lhs, ones_col[:],
                                         start=(kk == 0), stop=(kk == NS - 1))
                    recip = pstat.tile([P, 1], F32, name="recip", tag="recip")
                    nc.vector.reciprocal(recip[:], rs[:])
                    ob = pout.tile([P, D], F32, name="ob", tag="ob")
                    nc.vector.scalar_tensor_tensor(
                        ob[:, 0:EC], a0[:], recip[:], bv_bcast[:, 0:EC],
                        op0=ALU.mult, op1=ALU.add)
                    nc.vector.scalar_tensor_tensor(
                        ob[:, EC:D], a1[:], recip[:], bv_bcast[:, EC:D],
                        op0=ALU.mult, op1=ALU.add)
                    q_tile = qc * NH + qs
                    nc.sync.dma_start(
                        out_d[q_tile * P:(q_tile + 1) * P, :], ob[:])


def get_nc():
    if "nc" not in _CACHE:
        _CACHE["nc"] = _build_nc()
    return _CACHE["nc"]


def _prep_inputs(image_emb, text_emb, Wq, bq, Wk, bk, Wv, bv):
    bf = ml_dtypes.bfloat16
    xi = np.asarray(image_emb).astype(bf)   # [B, S, D]
    xt = np.asarray(text_emb).astype(bf)
    xiT = np.ascontiguousarray(xi.transpose(0, 2, 1))  # [B, D, S]
    xtT = np.ascontiguousarray(xt.transpose(0, 2, 1))
    wqt = np.ascontiguousarray(np.asarray(Wq).T).astype(bf)
    wkt = np.ascontiguousarray(np.asarray(Wk).T).astype(bf)
    wvt = np.ascontiguousarray(np.asarray(Wv).T).astype(bf)
    bq = np.ascontiguousarray(np.asarray(bq, dtype=np.float32).reshape(ND, P).T)
    bk = np.ascontiguousarray(np.asarray(bk, dtype=np.float32).reshape(ND, P).T)
    bv = np.asarray(bv, dtype=np.float32)
    in_maps = []
    for b in range(B):
        in_maps.append({
            "xiT": xiT[b], "xtT": xtT[b],
            "wqt": wqt, "wkt": wkt, "wvt": wvt,
            "bq": bq, "bk": bk, "bv": bv,
        })
    return in_maps


def run(image_emb, text_emb, Wq, bq, Wk, bk, Wv, bv, trace=False, **spmd_kwargs):
    nc = get_nc()
    in_maps = _prep_inputs(image_emb, text_emb, Wq, bq, Wk, bk, Wv, bv)
    res = run_bass_kernel_spmd(nc, in_maps, list(range(B)), trace=trace,
                               **spmd_kwargs)
    out = np.stack([res.results[b]["out"] for b in range(B)], axis=0)
    return out, res


def kernel(image_emb, text_emb, edge_index=None, Wq=None, bq=None, Wk=None,
           bk=None, Wv=None, bv=None, **_unused):
    out, _ = run(image_emb, text_emb, Wq, bq, Wk, bk, Wv, bv, trace=False)
    return out
